# revision 1
# baseline (speedup 1.0000x reference)
"""CaptionBase greedy GRU decode on 8 Trainium2 NeuronCores.

Sharding: proposal axis P=128 split 8 ways -> 16 proposals x 4 batch = 64
rows per core.  Each core runs the full 31-step greedy decode for its rows
and writes its [31, 64, V] logits slab; the host reassembles [B, P, 31, V].

Matmul precision: every matmul is a 3-pass hi/lo split
    h1(fp16) @ w1(fp16)  +  h1(bf16) @ w2(bf16)  +  h2(bf16) @ w1(bf16)
accumulated into one PSUM group, where h1 = fp16(h), h2 = h - h1,
w1 = fp16(W), w2 = W - w1.  Dropped term h2@w2 ~ 2^-22, so the result is
fp32-grade (~2e-6 relative) at 3 cycles/row instead of fp32's 4 -- this
matters because greedy argmax decisions must match the fp32 reference
exactly (tightest observed top-2 logit gap is ~8e-6 sigma).
"""

import numpy as np
import ml_dtypes

import concourse.bass as bass
import concourse.bacc as bacc
import concourse.mybir as mybir
from concourse.tile import TileContext
from concourse.masks import make_identity
from concourse.bass_utils import run_bass_kernel_spmd

B, P, T, E, F, H, V = 4, 128, 32, 300, 2048, 512, 3433
NSTEP = T - 1          # 31 decode steps
NCORES = 8
PL = P // NCORES       # 16 proposals per core
R = PL * B             # 64 rows per core
EP = 384               # E padded to 3*128
KE, KH, KF = 3, 4, 16  # k-chunks for E/H/F contractions
G3 = 3 * H             # 1536
NV = (V + 511) // 512  # 7 vocab chunks

f32 = mybir.dt.float32
f16 = mybir.dt.float16
bf16 = mybir.dt.bfloat16
u32 = mybir.dt.uint32
AFT = mybir.ActivationFunctionType

FP16_MIN_NORMAL = 2.0 ** -14
_CACHE = {}


def _split3(w):
    """fp32 array -> (w1 fp16, w2 bf16, w1b bf16) with w ~= w1 + w2.

    fp16 subnormals are pre-flushed on the host so the PE and numpy agree
    on w1; the bf16 residual absorbs whatever was flushed.
    """
    w = np.ascontiguousarray(w, dtype=np.float32)
    w1 = w.astype(np.float16)
    w1[np.abs(w1.astype(np.float32)) < FP16_MIN_NORMAL] = 0
    w2 = (w - w1.astype(np.float32)).astype(ml_dtypes.bfloat16)
    w1b = w1.astype(ml_dtypes.bfloat16)
    return w1, w2, w1b


def _pack_chunks(a, kchunks):
    """[kchunks*128, R] -> [128, kchunks*R] with chunk c at cols c*R:(c+1)*R."""
    k128, r = a.shape
    assert k128 == kchunks * 128
    out = np.empty((128, kchunks * r), dtype=a.dtype)
    for c in range(kchunks):
        out[:, c * r:(c + 1) * r] = a[c * 128:(c + 1) * 128]
    return np.ascontiguousarray(out)


def _build_program(nonzero_bias, nstep=NSTEP, debug=False):
    nc = bacc.Bacc("TRN2", target_bir_lowering=False)

    def din(name, shape, dt):
        return nc.dram_tensor(name, shape, dt, kind="ExternalInput")

    # Weight splits (moving operands), shared across cores.
    wih = [din(f"wih{i}", [EP, G3], d) for i, d in enumerate((f16, bf16, bf16))]
    whh = [din(f"whh{i}", [H, G3], d) for i, d in enumerate((f16, bf16, bf16))]
    wcl = [din(f"wcl{i}", [H, V], d) for i, d in enumerate((f16, bf16, bf16))]
    wmp = [din(f"wmp{i}", [F, H], d) for i, d in enumerate((f16, bf16, bf16))]
    # Stationary setup operands, packed [128, k*R]; obj differs per core.
    obj = [din(f"obj{i}", [128, KF * R], d) for i, d in enumerate((f16, bf16, bf16))]
    x0 = [din(f"x0{i}", [128, KE * R], d) for i, d in enumerate((f16, bf16, bf16))]
    emb = din("emb", [V, E], f32)
    biases = {}
    for bname, blen in (("b_rz", 2 * H), ("b_in", H), ("b_hn", H), ("b_cls", V),
                        ("b_map", H)):
        if nonzero_bias.get(bname):
            biases[bname] = din(bname, [1, blen], f32)
    out_dram = nc.dram_tensor("out", [nstep, R, V], f32, kind="ExternalOutput")
    dbg = {}
    if debug:
        for dn, shp in (("dbg_h0", [R, H]), ("dbg_rz", [R, 2 * H]),
                        ("dbg_in", [R, H]), ("dbg_hn", [R, H]),
                        ("dbg_h1", [R, H]), ("dbg_x1", [R, E])):
            dbg[dn] = nc.dram_tensor(dn, shp, f32, kind="ExternalOutput")

    with TileContext(nc) as tc:
        with (
            tc.tile_pool(name="const", bufs=1) as const,
            tc.tile_pool(name="wpool", bufs=1) as wpool,
            tc.tile_pool(name="state", bufs=1) as state,
            tc.tile_pool(name="work", bufs=2) as work,
            tc.tile_pool(name="psum", bufs=1, space="PSUM") as psum,
            tc.tile_pool(name="psum2", bufs=2, space="PSUM") as psum2,
        ):
            ident = const.tile([128, 128], f32)
            make_identity(nc, ident)

            bias_t = {}
            for bname, ap in biases.items():
                blen = ap.shape[1]
                bt = const.tile([R, blen], f32, name=f"{bname}_t")
                nc.sync.dma_start(out=bt, in_=ap.to_broadcast([R, blen]))
                bias_t[bname] = bt

            # Resident weight tiles: [128, G3] / [128, V] row-chunks.
            wih_t = [[wpool.tile([128, G3], w.dtype, name=f"wih{i}_{c}")
                      for c in range(KE)] for i, w in enumerate(wih)]
            whh_t = [[wpool.tile([128, G3], w.dtype, name=f"whh{i}_{c}")
                      for c in range(KH)] for i, w in enumerate(whh)]
            wcl_t = [[wpool.tile([128, V], w.dtype, name=f"wcl{i}_{c}")
                      for c in range(KH)] for i, w in enumerate(wcl)]
            for i in range(3):
                for c in range(KE):
                    nc.sync.dma_start(out=wih_t[i][c],
                                      in_=wih[i][c * 128:(c + 1) * 128, :])
                for c in range(KH):
                    nc.sync.dma_start(out=whh_t[i][c],
                                      in_=whh[i][c * 128:(c + 1) * 128, :])
                for c in range(KH):
                    nc.sync.dma_start(out=wcl_t[i][c],
                                      in_=wcl[i][c * 128:(c + 1) * 128, :])

            # Persistent transposed-state tiles ([128, k*R], chunk-major).
            xT = state.tile([128, KE * R], f32)
            xT1 = state.tile([128, KE * R], f16)
            xT2b = state.tile([128, KE * R], bf16)
            xT1b = state.tile([128, KE * R], bf16)
            nc.vector.memset(xT, 0.0)
            # x0 splits come pre-packed from the host.
            nc.sync.dma_start(out=xT1, in_=x0[0][:, :])
            nc.sync.dma_start(out=xT2b, in_=x0[1][:, :])
            nc.sync.dma_start(out=xT1b, in_=x0[2][:, :])

            # --- h0 = relu(obj_feats @ W_map), streaming W_map chunks.
            with tc.tile_pool(name="setup", bufs=3) as setup_pool:
                obj_t = [setup_pool.tile([128, KF * R], o.dtype, bufs=1,
                                         name=f"obj_t{i}")
                         for i, o in enumerate(obj)]
                for i in range(3):
                    nc.sync.dma_start(out=obj_t[i], in_=obj[i][:, :])
                h0_ps = psum.tile([R, H], f32, tag="nps")
                nmm = 3 * KF
                mi = 0
                # W_map streamed in 8 rounds of 2 k-chunks per version.
                for rd in range(8):
                    wm_t = [setup_pool.tile([128, 2 * H], w.dtype, tag=f"wm{i}",
                                            name=f"wm{i}_{rd}",
                                            bufs=(1 if debug else 2))
                            for i, w in enumerate(wmp)]
                    for i in range(3):
                        nc.gpsimd.dma_start(
                            out=wm_t[i][:].rearrange("p (a n) -> p a n", a=2),
                            in_=wmp[i][256 * rd:256 * (rd + 1), :].rearrange(
                                "(a p) n -> p a n", p=128))
                    for cc in range(2):
                        c = rd * 2 + cc
                        # pass pairing: (o1,w1), (o1b,w2), (o2b,w1b)
                        for ia, iw in ((0, 0), (2, 1), (1, 2)):
                            nc.tensor.matmul(h0_ps,
                                             lhsT=obj_t[ia][:, c * R:(c + 1) * R],
                                             rhs=wm_t[iw][:, cc * H:(cc + 1) * H],
                                             start=(mi == 0),
                                             stop=(mi == nmm - 1),
                                             skip_group_check=True)
                            mi += 1
                h_cur = work.tile([R, H], f32, tag="h")
                if "b_map" in bias_t:
                    nc.vector.tensor_add(h0_ps, h0_ps, bias_t["b_map"])
                nc.scalar.activation(h_cur, h0_ps, AFT.Relu)
                if debug:
                    nc.sync.dma_start(out=dbg["dbg_h0"][:, :], in_=h_cur)

            def transpose_split_h(h_ap):
                hT_ps = psum.tile([128, KH * R], f32, tag="trps")
                for c in range(KH):
                    nc.tensor.transpose(out=hT_ps[:, c * R:(c + 1) * R],
                                        in_=h_ap[:, c * 128:(c + 1) * 128],
                                        identity=ident[:R, :R])
                hT = work.tile([128, KH * R], f32, tag="hT")
                nc.scalar.copy(hT, hT_ps)
                hT1 = work.tile([128, KH * R], f16, tag="hT1")
                nc.vector.tensor_copy(hT1, hT)
                hT2b = work.tile([128, KH * R], bf16, tag="hT2b")
                nc.vector.tensor_sub(hT2b, hT, hT1)
                hT1b = work.tile([128, KH * R], bf16, tag="hT1b")
                nc.vector.tensor_copy(hT1b, hT1)
                return hT1, hT2b, hT1b

            hT1, hT2b, hT1b = transpose_split_h(h_cur)

            for t in range(nstep):
                # --- gate pre-activations ---------------------------------
                # rz_ps[:, :H] = i_r + h_r ; rz_ps[:, H:] = i_z + h_z
                # in_ps = i_n ; hn_ps = h_n
                rz_ps = psum.tile([R, 2 * H], f32, tag="rzps")
                in_ps = psum.tile([R, H], f32, tag="nps")
                hn_ps = psum.tile([R, H], f32, tag="hnps")
                # pass pairing: (h1,w1) fp16, (h1b,w2b) bf16, (h2b,w1b) bf16
                gi_passes = [(xT1, wih_t[0]), (xT1b, wih_t[1]), (xT2b, wih_t[2])]
                gh_passes = [(hT1, whh_t[0]), (hT1b, whh_t[1]), (hT2b, whh_t[2])]

                # rz halves: gh first (h ready before x arrives), then gi.
                for half in range(2):
                    sl = slice(half * H, (half + 1) * H)
                    n0 = half * H
                    total = 3 * KH + 3 * KE
                    mi = 0
                    for lh, rts in gh_passes:
                        for c in range(KH):
                            nc.tensor.matmul(
                                rz_ps[:, sl], lhsT=lh[:, c * R:(c + 1) * R],
                                rhs=rts[c][:, n0:n0 + H], start=(mi == 0),
                                stop=(mi == total - 1), skip_group_check=True)
                            mi += 1
                    for lh, rts in gi_passes:
                        for c in range(KE):
                            nc.tensor.matmul(
                                rz_ps[:, sl], lhsT=lh[:, c * R:(c + 1) * R],
                                rhs=rts[c][:, n0:n0 + H], start=False,
                                stop=(mi == total - 1), skip_group_check=True)
                            mi += 1
                mi = 0
                for lh, rts in gh_passes:
                    for c in range(KH):
                        nc.tensor.matmul(
                            hn_ps, lhsT=lh[:, c * R:(c + 1) * R],
                            rhs=rts[c][:, 2 * H:], start=(mi == 0),
                            stop=(mi == 3 * KH - 1), skip_group_check=True)
                        mi += 1
                mi = 0
                for lh, rts in gi_passes:
                    for c in range(KE):
                        nc.tensor.matmul(
                            in_ps, lhsT=lh[:, c * R:(c + 1) * R],
                            rhs=rts[c][:, 2 * H:], start=(mi == 0),
                            stop=(mi == 3 * KE - 1), skip_group_check=True)
                        mi += 1

                # --- gates -----------------------------------------------
                if "b_rz" in bias_t:
                    nc.vector.tensor_add(rz_ps, rz_ps, bias_t["b_rz"])
                rz_sb = work.tile([R, 2 * H], f32, tag="rzsb", bufs=1)
                if debug and t == 0:
                    dtmp = work.tile([R, 2 * H], f32, tag="dbgtmp", bufs=1)
                    nc.vector.tensor_copy(dtmp, rz_ps)
                    nc.sync.dma_start(out=dbg["dbg_rz"][:, :], in_=dtmp)
                    dtmp2 = work.tile([R, 2 * H], f32, tag="dbgtmp", bufs=1)
                    nc.vector.tensor_copy(dtmp2[:, :H], in_ps)
                    nc.vector.tensor_copy(dtmp2[:, H:], hn_ps)
                    nc.sync.dma_start(out=dbg["dbg_in"][:, :], in_=dtmp2[:, :H])
                    nc.sync.dma_start(out=dbg["dbg_hn"][:, :], in_=dtmp2[:, H:])
                nc.scalar.activation(rz_sb, rz_ps, AFT.Sigmoid)
                if "b_hn" in bias_t:
                    nc.vector.tensor_add(hn_ps, hn_ps, bias_t["b_hn"])
                if "b_in" in bias_t:
                    nc.vector.tensor_add(in_ps, in_ps, bias_t["b_in"])
                tmp = work.tile([R, H], f32, tag="tmp")
                nc.vector.tensor_mul(tmp, rz_sb[:, :H], hn_ps)      # r * h_n
                nc.vector.tensor_add(tmp, tmp, in_ps)               # + i_n
                n_sb = work.tile([R, H], f32, tag="n")
                nc.scalar.activation(n_sb, tmp, AFT.Tanh)
                d_sb = work.tile([R, H], f32, tag="d")
                nc.vector.tensor_sub(d_sb, h_cur, n_sb)             # h - n
                nc.vector.tensor_mul(d_sb, rz_sb[:, H:], d_sb)      # z * (h - n)
                h_new = work.tile([R, H], f32, tag="h")
                nc.vector.tensor_add(h_new, n_sb, d_sb)             # n + z*(h-n)
                h_cur = h_new
                if debug and t == 0:
                    nc.sync.dma_start(out=dbg["dbg_h1"][:, :], in_=h_new)

                hT1, hT2b, hT1b = transpose_split_h(h_cur)
                cls_passes = [(hT1, wcl_t[0]), (hT1b, wcl_t[1]), (hT2b, wcl_t[2])]

                # --- logits = h @ W_cls ----------------------------------
                logits = work.tile([R, V], f32, tag="logits", bufs=1)
                maxes = work.tile([R, 8 * NV], f32, tag="maxes")
                for v in range(NV):
                    n0 = v * 512
                    w = min(512, V - n0)
                    cls_ps = psum2.tile([R, 512], f32, tag="clsps")
                    mi = 0
                    for lh, rts in cls_passes:
                        for c in range(KH):
                            nc.tensor.matmul(
                                cls_ps[:, :w], lhsT=lh[:, c * R:(c + 1) * R],
                                rhs=rts[c][:, n0:n0 + w], start=(mi == 0),
                                stop=(mi == 3 * KH - 1), skip_group_check=True)
                            mi += 1
                    if "b_cls" in bias_t:
                        nc.vector.tensor_add(logits[:, n0:n0 + w], cls_ps[:, :w],
                                             bias_t["b_cls"][:, n0:n0 + w])
                    else:
                        nc.scalar.copy(logits[:, n0:n0 + w], cls_ps[:, :w])
                    nc.vector.max(maxes[:, v * 8:(v + 1) * 8], logits[:, n0:n0 + w])

                nc.sync.dma_start(out=out_dram[t, :, :], in_=logits)

                if t == nstep - 1:
                    continue
                # --- greedy argmax + next-token embedding gather ---------
                gmax = work.tile([R, 8], f32, tag="gmax")
                nc.vector.max(gmax, maxes)
                idx = work.tile([R, 8], u32, tag="idx")
                nc.vector.max_index(idx, gmax, logits)
                x_sb = work.tile([R, E], f32, tag="x")
                nc.gpsimd.indirect_dma_start(
                    out=x_sb, out_offset=None, in_=emb[:, :],
                    in_offset=bass.IndirectOffsetOnAxis(ap=idx[:, :1], axis=0))
                if debug and t == 0:
                    nc.sync.dma_start(out=dbg["dbg_x1"][:, :], in_=x_sb)
                xT_ps = psum.tile([128, KE * R], f32, tag="trps")
                for c in range(KE):
                    win = min(128, E - c * 128)
                    nc.tensor.transpose(out=xT_ps[:win, c * R:(c + 1) * R],
                                        in_=x_sb[:, c * 128:c * 128 + win],
                                        identity=ident[:R, :R])
                    nc.scalar.copy(xT[:win, c * R:(c + 1) * R],
                                   xT_ps[:win, c * R:(c + 1) * R])
                nc.vector.tensor_copy(xT1, xT)
                nc.vector.tensor_sub(xT2b, xT, xT1)
                nc.vector.tensor_copy(xT1b, xT1)

    nc.compile()
    return nc


def _prep_inputs(inputs):
    """Host-side layout prep: transposes, padding, hi/lo splits, packing."""
    word_embs = np.asarray(inputs["word_embs"], dtype=np.float32)
    obj_feats = np.asarray(inputs["obj_feats"], dtype=np.float32)
    W_map = np.asarray(inputs["W_map"], dtype=np.float32)
    W_ih = np.asarray(inputs["W_ih"], dtype=np.float32)
    W_hh = np.asarray(inputs["W_hh"], dtype=np.float32)
    W_cls = np.asarray(inputs["W_cls"], dtype=np.float32)
    emb_table = np.asarray(inputs["emb_table"], dtype=np.float32)
    b_ih = np.asarray(inputs["b_ih"], dtype=np.float32)
    b_hh = np.asarray(inputs["b_hh"], dtype=np.float32)
    b_cls = np.asarray(inputs["b_cls"], dtype=np.float32)
    b_map = np.asarray(inputs["b_map"], dtype=np.float32)

    wihT = np.zeros((EP, G3), np.float32)
    wihT[:E] = W_ih.T
    whhT = np.ascontiguousarray(W_hh.T)          # [H, 3H]

    shared = {}
    for name, w in (("wih", wihT), ("whh", whhT), ("wcl", W_cls), ("wmp", W_map)):
        for i, part in enumerate(_split3(w)):
            shared[f"{name}{i}"] = part

    # x0: SOS embedding, shared across proposals; column r of xT is row
    # r = p_local*B + b, i.e. x0T[:, r] = word_embs[r % B, 0, :].
    x0T = np.zeros((EP, R), np.float32)
    x0T[:E] = np.tile(word_embs[:, 0, :].T, (1, PL))
    for i, part in enumerate(_split3(x0T)):
        shared[f"x0{i}"] = _pack_chunks(part, KE)
    shared["emb"] = emb_table

    nonzero_bias = {}
    brz = (b_ih + b_hh)[:2 * H]
    for bname, val in (("b_rz", brz), ("b_in", b_ih[2 * H:]),
                       ("b_hn", b_hh[2 * H:]), ("b_cls", b_cls),
                       ("b_map", b_map)):
        if np.any(val):
            nonzero_bias[bname] = True
            shared[bname] = np.ascontiguousarray(val[None, :], dtype=np.float32)

    in_maps = []
    for c in range(NCORES):
        m = dict(shared)
        sl = obj_feats[:, c * PL:(c + 1) * PL]           # [B, PL, F]
        objT = np.ascontiguousarray(
            np.transpose(sl, (2, 1, 0)).reshape(F, R))   # col r = pl*B + b
        for i, part in enumerate(_split3(objT)):
            m[f"obj{i}"] = _pack_chunks(part, KF)
        in_maps.append(m)
    return in_maps, nonzero_bias


TRACE = False          # test-harness hook: set True to capture an NTFF trace
LAST_RESULTS = None


def kernel(**inputs):
    global LAST_RESULTS
    in_maps, nonzero_bias = _prep_inputs(inputs)
    key = tuple(sorted(nonzero_bias))
    if key not in _CACHE:
        _CACHE[key] = _build_program(nonzero_bias)
    nc = _CACHE[key]
    res = run_bass_kernel_spmd(nc, in_maps, core_ids=list(range(NCORES)),
                               trace=TRACE)
    LAST_RESULTS = res
    full = np.empty((B, P, NSTEP, V), np.float32)
    for c in range(NCORES):
        o = res.results[c]["out"].reshape(NSTEP, PL, B, V)
        full[:, c * PL:(c + 1) * PL] = np.transpose(o, (2, 1, 0, 3))
    return full



# revision 6
# speedup vs baseline: 1.4549x; 1.4549x over previous
"""CaptionBase greedy GRU decode on 8 Trainium2 NeuronCores.

Sharding: proposal axis P=128 split 8 ways -> 16 proposals x 4 batch = 64
rows per core.  Each core runs the full 31-step greedy decode for its rows
and writes its [31, 64, V] logits slab; the host reassembles [B, P, 31, V].

v2 structural changes over the 1.57ms baseline:
  * The input-side GRU matmuls are gone: x is always an embedding-table row,
    so gi = x @ W_ih.T + b_ih is precomputed on the host as a [V, 3H] fp32
    table and fetched per step with one 6KB/row indirect-DMA gather.  This
    also removes the per-step x gather + PE transpose + fp16/bf16 splits.
  * All matmuls are M=64 (rows), which leaves half the 128-wide PE array
    idle.  They now run column-tiled in pairs: PE tile (0,0) accumulates one
    full output region on PSUM partitions 0:64 while tile (0,64) accumulates
    a DIFFERENT region on 64:128 concurrently (~2x issue rate).  Pairs:
    r-gate with z-gate, cls chunk 2k with 2k+1, and h_n of step t+1 with cls
    chunk 6 of step t.  Upper-half consumers stay partition-aligned (scalar
    evac, DVE max) since engines cannot cross partitions; only z and the
    odd-chunk argmax candidates cross via small SBUF->SBUF DMAs off the
    critical path.
  * argmax is computed incrementally per 512-wide logits chunk (MAX8 +
    FIND_INDEX8 on [64,512] hidden under the next chunk's matmuls), then a
    dozen tiny [64,8] ops combine the 7 chunk candidates tie-safely.

Matmul precision: unchanged 3-pass hi/lo split
    h1(fp16) @ w1(fp16)  +  h1b(bf16) @ w2(bf16)  +  h2b(bf16) @ w1b(bf16)
(~2e-6 relative, fp32-grade) because greedy argmax decisions must match the
fp32 reference exactly (tightest observed top-2 logit gap ~8e-6 sigma).
"""

import numpy as np
import ml_dtypes

import concourse.bass as bass
import concourse.bacc as bacc
import concourse.mybir as mybir
from concourse.tile import TileContext
from concourse.masks import make_identity
from concourse.bass_utils import run_bass_kernel_spmd

B, P, T, E, F, H, V = 4, 128, 32, 300, 2048, 512, 3433
NSTEP = T - 1          # 31 decode steps
NCORES = 8
PL = P // NCORES       # 16 proposals per core
R = PL * B             # 64 rows per core
KH, KF = 4, 16         # k-chunks for H/F contractions
G3 = 3 * H             # 1536
NV = (V + 511) // 512  # 7 vocab chunks
# vocab chunks 0,2,4 accumulate on PE tile (0,0)/partitions 0:64 (slots
# 0..2 of the low logits tile); 1,3,5 and 6 on tile (0,64)/64:128.
LO_CH = (0, 2, 4)

f32 = mybir.dt.float32
f16 = mybir.dt.float16
bf16 = mybir.dt.bfloat16
u32 = mybir.dt.uint32
AFT = mybir.ActivationFunctionType
ALU = mybir.AluOpType

FP16_MIN_NORMAL = 2.0 ** -14
_CACHE = {}


def _split3(w):
    """fp32 array -> (w1 fp16, w2 bf16, w1b bf16) with w ~= w1 + w2."""
    w = np.ascontiguousarray(w, dtype=np.float32)
    w1 = w.astype(np.float16)
    w1[np.abs(w1.astype(np.float32)) < FP16_MIN_NORMAL] = 0
    w2 = (w - w1.astype(np.float32)).astype(ml_dtypes.bfloat16)
    w1b = w1.astype(ml_dtypes.bfloat16)
    return w1, w2, w1b


def _pack_chunks(a, kchunks):
    """[kchunks*128, R] -> [128, kchunks*R] with chunk c at cols c*R:(c+1)*R."""
    k128, r = a.shape
    assert k128 == kchunks * 128
    out = np.empty((128, kchunks * r), dtype=a.dtype)
    for c in range(kchunks):
        out[:, c * r:(c + 1) * r] = a[c * 128:(c + 1) * 128]
    return np.ascontiguousarray(out)


def _interleave(nc, mms_a, mms_b):
    """Emit two matmul arg-lists alternately so the PE overlaps the two
    column-tiles (concurrency requires alternating tile targets in issue
    order)."""
    for i in range(max(len(mms_a), len(mms_b))):
        if i < len(mms_a):
            nc.tensor.matmul(**mms_a[i])
        if i < len(mms_b):
            nc.tensor.matmul(**mms_b[i])


def _build_program(nonzero_bias, nstep=NSTEP):
    nc = bacc.Bacc("TRN2", target_bir_lowering=False)

    def din(name, shape, dt):
        return nc.dram_tensor(name, shape, dt, kind="ExternalInput")

    # Weight splits (moving operands), shared across cores.
    whh = [din(f"whh{i}", [H, G3], d) for i, d in enumerate((f16, bf16, bf16))]
    wcl = [din(f"wcl{i}", [H, V], d) for i, d in enumerate((f16, bf16, bf16))]
    wmp = [din(f"wmp{i}", [F, H], d) for i, d in enumerate((f16, bf16, bf16))]
    # Stationary setup operands, packed [128, k*R]; obj differs per core.
    obj = [din(f"obj{i}", [128, KF * R], d) for i, d in enumerate((f16, bf16, bf16))]
    gi_table = din("gi_table", [V, G3], f32)   # emb @ W_ih.T + b_ih
    gi0_d = din("gi0", [R, G3], f32)           # SOS token's gi, tiled to rows
    off_d = din("amax_off", [1, 8], f32)       # chunk offsets + col-7 poison
    biases = {}
    for bname, blen in (("b_rz", 2 * H), ("b_hn", H), ("b_cls", V),
                        ("b_map", H)):
        if nonzero_bias.get(bname):
            biases[bname] = din(bname, [1, blen], f32)
    out_dram = nc.dram_tensor("out", [nstep, R, V], f32, kind="ExternalOutput")

    with TileContext(nc) as tc:
        with (
            tc.tile_pool(name="const", bufs=1) as const,
            tc.tile_pool(name="wpool", bufs=1) as wpool,
            tc.tile_pool(name="state", bufs=1) as state,
            tc.tile_pool(name="work", bufs=2) as work,
            tc.tile_pool(name="psum", bufs=1, space="PSUM") as psum,
            tc.tile_pool(name="psum2", bufs=3, space="PSUM") as psum2,
        ):
            ident = const.tile([128, 128], f32)
            make_identity(nc, ident)

            off_t = const.tile([R, 8], f32, name="off_t")
            nc.sync.dma_start(out=off_t, in_=off_d[:, :].to_broadcast([R, 8]))

            bias_t = {}
            for bname, ap in biases.items():
                blen = ap.shape[1]
                bt = const.tile([R, blen], f32, name=f"{bname}_t")
                nc.sync.dma_start(out=bt, in_=ap[:, :].to_broadcast([R, blen]))
                bias_t[bname] = bt

            # Resident weight tiles: [128, G3] / [128, V] row-chunks.
            whh_t = [[wpool.tile([128, G3], w.dtype, name=f"whh{i}_{c}")
                      for c in range(KH)] for i, w in enumerate(whh)]
            wcl_t = [[wpool.tile([128, V], w.dtype, name=f"wcl{i}_{c}")
                      for c in range(KH)] for i, w in enumerate(wcl)]
            for i in range(3):
                for c in range(KH):
                    nc.sync.dma_start(out=whh_t[i][c],
                                      in_=whh[i][c * 128:(c + 1) * 128, :])
                for c in range(KH):
                    nc.sync.dma_start(out=wcl_t[i][c],
                                      in_=wcl[i][c * 128:(c + 1) * 128, :])

            # Persistent argmax-combine tiles (their memsets must survive).
            cm8 = state.tile([R, 8], f32, name="cm8")
            lidxf = state.tile([R, 8], f32, name="lidxf")
            nc.vector.memset(cm8, -3.0e38)
            nc.vector.memset(lidxf, 0.0)

            gi_cur = work.tile([R, G3], f32, tag="gi")
            nc.sync.dma_start(out=gi_cur, in_=gi0_d[:, :])

            # --- h0 = relu(obj_feats @ W_map): even k-chunks on tile (0,0),
            # odd on (0,64); one cross-partition evac DMA folds the halves.
            with tc.tile_pool(name="setup", bufs=3) as setup_pool:
                obj_t = [setup_pool.tile([128, KF * R], o.dtype, bufs=1,
                                         name=f"obj_t{i}")
                         for i, o in enumerate(obj)]
                for i in range(3):
                    nc.sync.dma_start(out=obj_t[i], in_=obj[i][:, :])
                h0_ps = psum2.tile([128, 512], f32, tag="clsps")
                nhalf = 3 * KF // 2
                mi = [0, 0]
                for rd in range(8):
                    wm_t = [setup_pool.tile([128, 2 * H], w.dtype, tag=f"wm{i}",
                                            name=f"wm{i}_{rd}", bufs=2)
                            for i, w in enumerate(wmp)]
                    for i in range(3):
                        nc.gpsimd.dma_start(
                            out=wm_t[i][:].rearrange("p (a n) -> p a n", a=2),
                            in_=wmp[i][256 * rd:256 * (rd + 1), :].rearrange(
                                "(a p) n -> p a n", p=128))
                    # pass pairing: (o1,w1), (o1b,w2), (o2b,w1b)
                    for ia, iw in ((0, 0), (2, 1), (1, 2)):
                        for cc in range(2):
                            c = rd * 2 + cc
                            nc.tensor.matmul(
                                h0_ps[64 * cc:64 * cc + 64, :],
                                lhsT=obj_t[ia][:, c * R:(c + 1) * R],
                                rhs=wm_t[iw][:, cc * H:(cc + 1) * H],
                                start=(mi[cc] == 0),
                                stop=(mi[cc] == nhalf - 1),
                                skip_group_check=True,
                                tile_position=(0, 64 * cc))
                            mi[cc] += 1
                h0_hi = work.tile([128, H], f32, tag="zhi")
                nc.scalar.copy(h0_hi[64:128, :], h0_ps[64:128, :])
                h0_lo = work.tile([R, H], f32, tag="zlo")
                nc.sync.dma_start(out=h0_lo, in_=h0_hi[64:128, :])
                h0_sb = work.tile([R, H], f32, tag="tmp")
                nc.vector.tensor_add(h0_sb, h0_ps[0:64, :], h0_lo)
                if "b_map" in bias_t:
                    nc.vector.tensor_add(h0_sb, h0_sb, bias_t["b_map"])
                h_cur = work.tile([R, H], f32, tag="h")
                nc.scalar.activation(h_cur, h0_sb, AFT.Relu)

            def transpose_split_h(h_ap):
                hT_ps = psum2.tile([128, KH * R], f32, tag="trps", bufs=1)
                for c in range(KH):
                    nc.tensor.transpose(out=hT_ps[:, c * R:(c + 1) * R],
                                        in_=h_ap[:, c * 128:(c + 1) * 128],
                                        identity=ident[:R, :R])
                hT = work.tile([128, KH * R], f32, tag="hT")
                nc.scalar.copy(hT, hT_ps)
                hT1 = work.tile([128, KH * R], f16, tag="hT1")
                nc.vector.tensor_copy(hT1, hT)
                hT2b = work.tile([128, KH * R], bf16, tag="hT2b")
                nc.vector.tensor_sub(hT2b, hT, hT1)
                hT1b = work.tile([128, KH * R], bf16, tag="hT1b")
                nc.vector.tensor_copy(hT1b, hT1)
                return hT1, hT2b, hT1b

            def gate_mms(split, wt, ps, n0, w, half, ngrp=12):
                """12 matmul-arg dicts accumulating one [64, w] region of
                h @ W[:, n0:n0+w] on PE tile (0, 64*half)."""
                hT1, hT2b, hT1b = split
                out = []
                mi = 0
                for lh, rts in ((hT1, wt[0]), (hT1b, wt[1]), (hT2b, wt[2])):
                    for c in range(KH):
                        out.append(dict(
                            out=ps[64 * half:64 * half + 64, :w],
                            lhsT=lh[:, c * R:(c + 1) * R],
                            rhs=rts[c][:, n0:n0 + w],
                            start=(mi == 0), stop=(mi == ngrp - 1),
                            skip_group_check=True,
                            tile_position=(0, 64 * half)))
                        mi += 1
                return out

            hsplit = transpose_split_h(h_cur)

            # h_n of step 0 has no cls chunk to pair with; emit it solo.
            hn_ps = psum.tile([128, H], f32, tag="hnps")
            _interleave(nc, gate_mms(hsplit, whh_t, hn_ps, 2 * H, H, 0), [])

            for t in range(nstep):
                # --- r (tile 0,0 / psum 0:64) paired with z (0,64 / 64:128).
                rz_ps = psum.tile([128, H], f32, tag="rzps")
                _interleave(nc,
                            gate_mms(hsplit, whh_t, rz_ps, 0, H, 0),
                            gate_mms(hsplit, whh_t, rz_ps, H, H, 1))

                # z is needed late; evacuate + cross to partitions 0:64 now.
                z_hi = work.tile([128, H], f32, tag="zhi")
                nc.scalar.copy(z_hi[64:128, :], rz_ps[64:128, :])
                z_lo = work.tile([R, H], f32, tag="zlo")
                nc.sync.dma_start(out=z_lo, in_=z_hi[64:128, :])

                # --- gates -----------------------------------------------
                r_sum = work.tile([R, H], f32, tag="rsum")
                nc.vector.tensor_add(r_sum, rz_ps[0:64, :], gi_cur[:, :H])
                if "b_rz" in bias_t:
                    nc.vector.tensor_add(r_sum, r_sum, bias_t["b_rz"][:, :H])
                r_sb = work.tile([R, H], f32, tag="rsb")
                nc.scalar.activation(r_sb, r_sum, AFT.Sigmoid)
                tmp = work.tile([R, H], f32, tag="tmp")
                if "b_hn" in bias_t:
                    hn_sum = work.tile([R, H], f32, tag="hnsum")
                    nc.vector.tensor_add(hn_sum, hn_ps[0:64, :],
                                         bias_t["b_hn"])
                    nc.vector.tensor_mul(tmp, r_sb, hn_sum)     # r * h_n
                else:
                    nc.vector.tensor_mul(tmp, r_sb, hn_ps[0:64, :])
                nc.vector.tensor_add(tmp, tmp, gi_cur[:, 2 * H:])   # + i_n
                n_sb = work.tile([R, H], f32, tag="n")
                nc.scalar.activation(n_sb, tmp, AFT.Tanh)
                z_sum = work.tile([R, H], f32, tag="zsum")
                nc.vector.tensor_add(z_sum, z_lo, gi_cur[:, H:2 * H])
                if "b_rz" in bias_t:
                    nc.vector.tensor_add(z_sum, z_sum, bias_t["b_rz"][:, H:])
                z_sb = work.tile([R, H], f32, tag="zsb")
                nc.scalar.activation(z_sb, z_sum, AFT.Sigmoid)
                d_sb = work.tile([R, H], f32, tag="d")
                nc.vector.tensor_sub(d_sb, h_cur, n_sb)             # h - n
                nc.vector.tensor_mul(d_sb, z_sb, d_sb)              # z * (h - n)
                h_new = work.tile([R, H], f32, tag="h")
                nc.vector.tensor_add(h_new, n_sb, d_sb)             # n + z*(h-n)
                h_cur = h_new

                hsplit = transpose_split_h(h_cur)

                # --- logits chunks: pairs (0,1) (2,3) (4,5) then 6 paired
                # with h_n of step t+1.  Evac + DMA + argmax per chunk on its
                # native partition half.
                log_lo = work.tile([128, 3 * 512], f32, tag="loglo", bufs=1)
                log_hi = work.tile([128, 4 * 512], f32, tag="loghi", bufs=1)
                mx = work.tile([128, 4 * 8], f32, tag="mx")
                li = work.tile([128, 4 * 8], u32, tag="li")

                def evac_chunk(v, slot, ps, t=t, log_lo=log_lo, log_hi=log_hi,
                               mx=mx, li=li):
                    n0 = v * 512
                    w = min(512, V - n0)
                    half = 0 if v in LO_CH else 1
                    p0 = 64 * half
                    lg = log_lo if half == 0 else log_hi
                    sl = lg[p0:p0 + 64, slot * 512:slot * 512 + w]
                    nc.scalar.copy(sl, ps[p0:p0 + 64, :w])
                    if "b_cls" in bias_t:
                        nc.vector.tensor_add(sl, sl,
                                             bias_t["b_cls"][:, n0:n0 + w])
                    nc.sync.dma_start(out=out_dram[t, :, n0:n0 + w], in_=sl)
                    nc.vector.max(mx[p0:p0 + 64, slot * 8:slot * 8 + 8], sl)
                    if t < nstep - 1:
                        nc.vector.max_index(
                            li[p0:p0 + 64, slot * 8:slot * 8 + 8],
                            mx[p0:p0 + 64, slot * 8:slot * 8 + 8], sl)

                def cls_mms(v, ps):
                    half = 0 if v in LO_CH else 1
                    n0 = v * 512
                    return gate_mms(hsplit, wcl_t, ps, n0, min(512, V - n0),
                                    half)

                for pr in range(3):
                    ps = psum2.tile([128, 512], f32, tag="clsps")
                    _interleave(nc, cls_mms(2 * pr, ps), cls_mms(2 * pr + 1, ps))
                    evac_chunk(2 * pr, pr, ps)
                    evac_chunk(2 * pr + 1, pr, ps)

                # chunk 6 (tile 0,64) ∥ h_n of step t+1 (tile 0,0)
                c6_ps = psum2.tile([128, 512], f32, tag="clsps")
                if t < nstep - 1:
                    hn_ps = psum.tile([128, H], f32, tag="hnps")
                    _interleave(nc,
                                gate_mms(hsplit, whh_t, hn_ps, 2 * H, H, 0),
                                cls_mms(6, c6_ps))
                else:
                    _interleave(nc, cls_mms(6, c6_ps), [])
                evac_chunk(6, 3, c6_ps)

                if t == nstep - 1:
                    continue
                # --- combine the 7 per-chunk argmax candidates ------------
                # (tie -> lowest global index, matching jnp.argmax).
                mx_x = work.tile([R, 4 * 8], f32, tag="mxx")
                li_x = work.tile([R, 4 * 8], u32, tag="lix")
                nc.sync.dma_start(out=mx_x, in_=mx[64:128, :])
                nc.sync.dma_start(out=li_x, in_=li[64:128, :])
                m_lo = mx[0:64, :].rearrange("p (v e) -> p v e", e=8)
                m_hi = mx_x[:, :].rearrange("p (v e) -> p v e", e=8)
                l_lo = li[0:64, :].rearrange("p (v e) -> p v e", e=8)
                l_hi = li_x[:, :].rearrange("p (v e) -> p v e", e=8)
                c2 = cm8[:, :].rearrange("p (v e) -> p v e", e=1)
                f2 = lidxf[:, :].rearrange("p (v e) -> p v e", e=1)
                nc.vector.tensor_copy(c2[:, 0:3, :], m_lo[:, 0:3, 0:1])
                nc.vector.tensor_copy(c2[:, 3:7, :], m_hi[:, 0:4, 0:1])
                nc.vector.tensor_copy(f2[:, 0:3, :], l_lo[:, 0:3, 0:1])
                nc.vector.tensor_copy(f2[:, 3:7, :], l_hi[:, 0:4, 0:1])
                g8 = work.tile([R, 8], f32, tag="g8")
                nc.vector.max(g8, cm8)
                mask = work.tile([R, 8], f32, tag="mask")
                nc.vector.tensor_scalar(mask, cm8, g8[:, 0:1], None, ALU.is_ge)
                pen = work.tile([R, 8], f32, tag="pen")
                nc.vector.tensor_scalar(pen, mask, -1.0e9, 1.0e9,
                                        ALU.mult, ALU.add)
                cand = work.tile([R, 8], f32, tag="cand")
                nc.vector.tensor_add(cand, lidxf, off_t)
                nc.vector.tensor_add(cand, cand, pen)
                nc.vector.tensor_scalar_mul(cand, cand, -1.0)
                g8b = work.tile([R, 8], f32, tag="g8b")
                nc.vector.max(g8b, cand)
                idxf = work.tile([R, 8], f32, tag="idxf")
                nc.vector.tensor_scalar_mul(idxf[:, 0:1], g8b[:, 0:1], -1.0)
                idx = work.tile([R, 8], u32, tag="idx")
                nc.vector.tensor_copy(idx[:, 0:1], idxf[:, 0:1])

                gi_cur = work.tile([R, G3], f32, tag="gi")
                nc.gpsimd.indirect_dma_start(
                    out=gi_cur, out_offset=None, in_=gi_table[:, :],
                    in_offset=bass.IndirectOffsetOnAxis(ap=idx[:, :1], axis=0))

    nc.compile()
    return nc


def _prep_inputs(inputs):
    """Host-side layout prep: transposes, padding, hi/lo splits, packing."""
    word_embs = np.asarray(inputs["word_embs"], dtype=np.float32)
    obj_feats = np.asarray(inputs["obj_feats"], dtype=np.float32)
    W_map = np.asarray(inputs["W_map"], dtype=np.float32)
    W_ih = np.asarray(inputs["W_ih"], dtype=np.float32)
    W_hh = np.asarray(inputs["W_hh"], dtype=np.float32)
    W_cls = np.asarray(inputs["W_cls"], dtype=np.float32)
    emb_table = np.asarray(inputs["emb_table"], dtype=np.float32)
    b_ih = np.asarray(inputs["b_ih"], dtype=np.float32)
    b_hh = np.asarray(inputs["b_hh"], dtype=np.float32)
    b_cls = np.asarray(inputs["b_cls"], dtype=np.float32)
    b_map = np.asarray(inputs["b_map"], dtype=np.float32)

    whhT = np.ascontiguousarray(W_hh.T)          # [H, 3H]

    shared = {}
    for name, w in (("whh", whhT), ("wcl", W_cls), ("wmp", W_map)):
        for i, part in enumerate(_split3(w)):
            shared[f"{name}{i}"] = part

    # gi lookup table: exact x-side gate pre-activations per vocab token.
    gi_table = (emb_table.astype(np.float64) @ W_ih.T.astype(np.float64)
                + b_ih.astype(np.float64)).astype(np.float32)
    shared["gi_table"] = np.ascontiguousarray(gi_table)
    gi0_row = (word_embs[:, 0, :].astype(np.float64) @ W_ih.T.astype(np.float64)
               + b_ih.astype(np.float64)).astype(np.float32)   # [B, 3H]
    shared["gi0"] = np.ascontiguousarray(np.tile(gi0_row, (PL, 1)))
    # combine order: low chunks 0,2,4 then high 1,3,5,6, then poison.
    shared["amax_off"] = np.array(
        [[0.0, 1024.0, 2048.0, 512.0, 1536.0, 2560.0, 3072.0, 1.0e9]],
        dtype=np.float32)

    nonzero_bias = {}
    for bname, val in (("b_rz", b_hh[:2 * H]), ("b_hn", b_hh[2 * H:]),
                       ("b_cls", b_cls), ("b_map", b_map)):
        if np.any(val):
            nonzero_bias[bname] = True
            shared[bname] = np.ascontiguousarray(val[None, :], dtype=np.float32)

    in_maps = []
    for c in range(NCORES):
        m = dict(shared)
        sl = obj_feats[:, c * PL:(c + 1) * PL]           # [B, PL, F]
        objT = np.ascontiguousarray(
            np.transpose(sl, (2, 1, 0)).reshape(F, R))   # col r = pl*B + b
        for i, part in enumerate(_split3(objT)):
            m[f"obj{i}"] = _pack_chunks(part, KF)
        in_maps.append(m)
    return in_maps, nonzero_bias


TRACE = False          # test-harness hook: set True to capture an NTFF trace
LAST_RESULTS = None


def kernel(**inputs):
    global LAST_RESULTS
    in_maps, nonzero_bias = _prep_inputs(inputs)
    key = tuple(sorted(nonzero_bias))
    if key not in _CACHE:
        _CACHE[key] = _build_program(nonzero_bias)
    nc = _CACHE[key]
    res = run_bass_kernel_spmd(nc, in_maps, core_ids=list(range(NCORES)),
                               trace=TRACE)
    LAST_RESULTS = res
    full = np.empty((B, P, NSTEP, V), np.float32)
    for c in range(NCORES):
        o = res.results[c]["out"].reshape(NSTEP, PL, B, V)
        full[:, c * PL:(c + 1) * PL] = np.transpose(o, (2, 1, 0, 3))
    return full


# revision 13
# speedup vs baseline: 1.5569x; 1.0701x over previous
"""CaptionBase greedy GRU decode on 8 Trainium2 NeuronCores.

Sharding: proposal axis P=128 split 8 ways -> 16 proposals x 4 batch = 64
rows per core.  Each core runs the full 31-step greedy decode for its rows
and writes its [31, 64, V] logits slab; the host reassembles [B, P, 31, V].

Structure (v4):
  * x-side GRU matmuls are a host-precomputed [V, 3H] fp32 table
    (gi = emb @ W_ih.T + b_ih) fetched per step by indirect-DMA gather,
    which also removes the per-step x gather + transpose + dtype splits.
  * Weights stream as float32r (full fp32 operand bits at 1 col/cycle for
    N=512), so a single pass replaces the baseline's 3-pass fp16/bf16 hi/lo
    split -- 3x less PE streaming.  float32r matmuls must write PSUM
    partitions 0:64 (the 64:128 column-tile fails the ISA dst check), so
    all matmuls run untiled with M=64.
  * Greedy argmax decisions must match the fp32 reference exactly (top-2
    logit gaps down to ~8e-6 sigma); float32r's fp32-grade accumulation is
    verified empirically by the harness argmax-mismatch count.
  * argmax is computed incrementally per 512-wide logits chunk, hidden
    under the next chunk's matmuls.  Odd chunks are DMA-staged to SBUF
    partitions 64:128 so one MAX8 + FIND_INDEX8 [128, 512] covers two
    chunks using all DVE lanes; a dozen tiny [64,8] ops then combine the 7
    chunk candidates tie-safely (tie -> lowest index, matching jnp.argmax).
"""

import numpy as np

import concourse.bass as bass
import concourse.bacc as bacc
import concourse.mybir as mybir
from concourse.tile import TileContext
from concourse.masks import make_identity
from concourse.bass_utils import run_bass_kernel_spmd

B, P, T, E, F, H, V = 4, 128, 32, 300, 2048, 512, 3433
NSTEP = T - 1          # 31 decode steps
NCORES = 8
PL = P // NCORES       # 16 proposals per core
R = PL * B             # 64 rows per core
KH, KF = 4, 16         # k-chunks for H/F contractions
G3 = 3 * H             # 1536
NV = (V + 511) // 512  # 7 vocab chunks
VP = NV * 512          # V padded so every fp32r matmul has even N=512
LO_CH = (0, 2, 4)      # chunks kept on partitions 0:64 (1,3,5,6 staged hi)

f32 = mybir.dt.float32
f32r = mybir.dt.float32r
u32 = mybir.dt.uint32
AFT = mybir.ActivationFunctionType
ALU = mybir.AluOpType

_CACHE = {}


def _pack_chunks(a, kchunks):
    """[kchunks*128, R] -> [128, kchunks*R] with chunk c at cols c*R:(c+1)*R."""
    k128, r = a.shape
    assert k128 == kchunks * 128
    out = np.empty((128, kchunks * r), dtype=a.dtype)
    for c in range(kchunks):
        out[:, c * r:(c + 1) * r] = a[c * 128:(c + 1) * 128]
    return np.ascontiguousarray(out)


def _build_program(nonzero_bias, nstep=NSTEP):
    nc = bacc.Bacc("TRN2", target_bir_lowering=False)

    def din(name, shape, dt):
        return nc.dram_tensor(name, shape, dt, kind="ExternalInput")

    # Weights (moving operands, float32r), shared across cores.
    whh_d = din("whh", [H, G3], f32r)
    wcl_d = din("wcl", [H, VP], f32r)
    wmp_d = din("wmp", [F, H], f32r)
    # Stationary setup operand, packed [128, KF*R]; differs per core.
    obj_d = din("obj", [128, KF * R], f32r)
    gi_table = din("gi_table", [V, G3], f32)   # emb @ W_ih.T + b_ih
    gi0_d = din("gi0", [R, G3], f32)           # SOS token's gi, tiled to rows
    off_d = din("amax_off", [1, 8], f32)       # chunk offsets + col-7 poison
    biases = {}
    for bname, blen in (("b_rz", 2 * H), ("b_hn", H), ("b_cls", V),
                        ("b_map", H)):
        if nonzero_bias.get(bname):
            biases[bname] = din(bname, [1, blen], f32)
    out_dram = nc.dram_tensor("out", [nstep, R, V], f32, kind="ExternalOutput")

    with TileContext(nc) as tc:
        with (
            tc.tile_pool(name="const", bufs=1) as const,
            tc.tile_pool(name="wpool", bufs=1) as wpool,
            tc.tile_pool(name="state", bufs=1) as state,
            tc.tile_pool(name="work", bufs=2) as work,
            tc.tile_pool(name="psum", bufs=1, space="PSUM") as psum,
            tc.tile_pool(name="psum2", bufs=3, space="PSUM") as psum2,
        ):
            ident = const.tile([128, 128], f32)
            make_identity(nc, ident)

            off_t = const.tile([R, 8], f32, name="off_t")
            nc.sync.dma_start(out=off_t, in_=off_d[:, :].to_broadcast([R, 8]))

            bias_t = {}
            for bname, ap in biases.items():
                blen = ap.shape[1]
                bt = const.tile([R, blen], f32, name=f"{bname}_t")
                nc.sync.dma_start(out=bt, in_=ap[:, :].to_broadcast([R, blen]))
                bias_t[bname] = bt

            # Resident weight tiles, [128, .] row-chunks.
            whh_t = [wpool.tile([128, G3], f32r, name=f"whh_{c}")
                     for c in range(KH)]
            wcl_t = [wpool.tile([128, VP], f32r, name=f"wcl_{c}")
                     for c in range(KH)]
            for c in range(KH):
                nc.sync.dma_start(out=whh_t[c],
                                  in_=whh_d[c * 128:(c + 1) * 128, :])
                nc.sync.dma_start(out=wcl_t[c],
                                  in_=wcl_d[c * 128:(c + 1) * 128, :])

            # Persistent argmax-combine tiles (their memsets must survive).
            cm8 = state.tile([R, 8], f32, name="cm8")
            lidxf = state.tile([R, 8], f32, name="lidxf")
            nc.vector.memset(cm8, -3.0e38)
            nc.vector.memset(lidxf, 0.0)

            gi_cur = work.tile([R, G3], f32, tag="gi")
            nc.sync.dma_start(out=gi_cur, in_=gi0_d[:, :])

            # --- h0 = relu(obj_feats @ W_map), streaming W_map. -----------
            with tc.tile_pool(name="setup", bufs=3) as setup_pool:
                obj_t = setup_pool.tile([128, KF * R], f32r, bufs=1,
                                        name="obj_t")
                nc.sync.dma_start(out=obj_t, in_=obj_d[:, :])
                h0_ps = psum2.tile([64, 512], f32, tag="clsps")
                for rd in range(8):
                    wm_t = setup_pool.tile([128, 2 * H], f32r, tag="wm",
                                           name=f"wm_{rd}", bufs=2)
                    nc.gpsimd.dma_start(
                        out=wm_t[:].rearrange("p (a n) -> p a n", a=2),
                        in_=wmp_d[256 * rd:256 * (rd + 1), :].rearrange(
                            "(a p) n -> p a n", p=128))
                    for cc in range(2):
                        c = rd * 2 + cc
                        nc.tensor.matmul(
                            h0_ps[:, :],
                            lhsT=obj_t[:, c * R:(c + 1) * R],
                            rhs=wm_t[:, cc * H:(cc + 1) * H],
                            start=(c == 0), stop=(c == KF - 1),
                            skip_group_check=True)
                h_cur = work.tile([R, H], f32, tag="h")
                if "b_map" in bias_t:
                    h0_sb = work.tile([R, H], f32, tag="tmp")
                    nc.vector.tensor_add(h0_sb, h0_ps, bias_t["b_map"])
                    nc.scalar.activation(h_cur, h0_sb, AFT.Relu)
                else:
                    nc.scalar.activation(h_cur, h0_ps, AFT.Relu)

            def transpose_h(h_ap):
                hT_ps = psum2.tile([128, KH * R], f32, tag="trps", bufs=1)
                for c in range(KH):
                    nc.tensor.transpose(out=hT_ps[:, c * R:(c + 1) * R],
                                        in_=h_ap[:, c * 128:(c + 1) * 128],
                                        identity=ident[:R, :R])
                hT = work.tile([128, KH * R], f32r, tag="hT")
                nc.scalar.copy(hT, hT_ps)
                return hT

            def region_mms(hT, wt, out_ap, n0, w):
                """KH matmuls accumulating h @ W[:, n0:n0+w] into out_ap."""
                for c in range(KH):
                    nc.tensor.matmul(
                        out_ap, lhsT=hT[:, c * R:(c + 1) * R],
                        rhs=wt[c][:, n0:n0 + w],
                        start=(c == 0), stop=(c == KH - 1),
                        skip_group_check=True)

            hT = transpose_h(h_cur)

            for t in range(nstep):
                # --- gate pre-activations (h side), all on partitions 0:64.
                rz_ps = psum.tile([64, 2 * H], f32, tag="rzps")
                hn_ps = psum.tile([64, H], f32, tag="hnps")
                region_mms(hT, whh_t, rz_ps[:, 0:H], 0, H)
                region_mms(hT, whh_t, rz_ps[:, H:2 * H], H, H)
                region_mms(hT, whh_t, hn_ps[:, :], 2 * H, H)

                # --- gates -----------------------------------------------
                r_sum = work.tile([R, H], f32, tag="rsum")
                nc.vector.tensor_add(r_sum, rz_ps[:, 0:H], gi_cur[:, :H])
                if "b_rz" in bias_t:
                    nc.vector.tensor_add(r_sum, r_sum, bias_t["b_rz"][:, :H])
                r_sb = work.tile([R, H], f32, tag="rsb")
                nc.scalar.activation(r_sb, r_sum, AFT.Sigmoid)
                tmp = work.tile([R, H], f32, tag="tmp")
                if "b_hn" in bias_t:
                    hn_sum = work.tile([R, H], f32, tag="hnsum")
                    nc.vector.tensor_add(hn_sum, hn_ps, bias_t["b_hn"])
                    nc.vector.tensor_mul(tmp, r_sb, hn_sum)     # r * h_n
                else:
                    nc.vector.tensor_mul(tmp, r_sb, hn_ps)
                nc.vector.tensor_add(tmp, tmp, gi_cur[:, 2 * H:])   # + i_n
                n_sb = work.tile([R, H], f32, tag="n")
                nc.scalar.activation(n_sb, tmp, AFT.Tanh)
                z_sum = work.tile([R, H], f32, tag="zsum")
                nc.vector.tensor_add(z_sum, rz_ps[:, H:2 * H],
                                     gi_cur[:, H:2 * H])
                if "b_rz" in bias_t:
                    nc.vector.tensor_add(z_sum, z_sum, bias_t["b_rz"][:, H:])
                z_sb = work.tile([R, H], f32, tag="zsb")
                nc.scalar.activation(z_sb, z_sum, AFT.Sigmoid)
                d_sb = work.tile([R, H], f32, tag="d")
                nc.vector.tensor_sub(d_sb, h_cur, n_sb)             # h - n
                nc.vector.tensor_mul(d_sb, z_sb, d_sb)              # z * (h - n)
                h_new = work.tile([R, H], f32, tag="h")
                nc.vector.tensor_add(h_new, n_sb, d_sb)             # n + z*(h-n)
                h_cur = h_new

                hT = transpose_h(h_cur)

                # --- logits chunks.  Even chunks evac to log[0:64, slot];
                # odd chunks evac to a staging tile then DMA to
                # log[64:128, slot] so one MAX8/FIND_INDEX covers the pair.
                log = work.tile([128, 4 * 512], f32, tag="log", bufs=1)
                mx = work.tile([128, 4 * 8], f32, tag="mx")
                li = work.tile([128, 4 * 8], u32, tag="li")

                def amax_slot(slot, p, wv, t=t, log=log, mx=mx, li=li):
                    nc.vector.max(mx[p, slot * 8:slot * 8 + 8],
                                  log[p, slot * 512:slot * 512 + wv])
                    if t < nstep - 1:
                        nc.vector.max_index(
                            li[p, slot * 8:slot * 8 + 8],
                            mx[p, slot * 8:slot * 8 + 8],
                            log[p, slot * 512:slot * 512 + wv])

                for v in range(NV):
                    n0 = v * 512
                    w = min(512, V - n0)
                    slot = v // 2
                    ps = psum2.tile([64, 512], f32, tag="clsps")
                    region_mms(hT, wcl_t, ps[:, :], n0, 512)
                    if v in LO_CH:
                        sl = log[0:64, slot * 512:slot * 512 + w]
                        nc.scalar.copy(sl, ps[:, :w])
                        if "b_cls" in bias_t:
                            nc.vector.tensor_add(sl, sl,
                                                 bias_t["b_cls"][:, n0:n0 + w])
                        nc.sync.dma_start(out=out_dram[t, :, n0:n0 + w],
                                          in_=sl)
                    else:
                        st = work.tile([R, 512], f32, tag="stage")
                        nc.scalar.copy(st[:, :w], ps[:, :w])
                        if "b_cls" in bias_t:
                            nc.vector.tensor_add(st[:, :w], st[:, :w],
                                                 bias_t["b_cls"][:, n0:n0 + w])
                        nc.sync.dma_start(out=out_dram[t, :, n0:n0 + w],
                                          in_=st[:, :w])
                        nc.sync.dma_start(
                            out=log[64:128, slot * 512:slot * 512 + w],
                            in_=st[:, :w])
                        if v < 6:
                            # pair complete -> [128, 512] argmax reduction
                            amax_slot(slot, slice(0, 128), 512)
                # chunk 6 rides alone in slot 3's hi half.
                amax_slot(3, slice(64, 128), 361)

                if t == nstep - 1:
                    continue
                # --- combine the 7 per-chunk argmax candidates ------------
                # (tie -> lowest global index, matching jnp.argmax).
                mx_x = work.tile([R, 4 * 8], f32, tag="mxx")
                li_x = work.tile([R, 4 * 8], u32, tag="lix")
                nc.sync.dma_start(out=mx_x, in_=mx[64:128, :])
                nc.sync.dma_start(out=li_x, in_=li[64:128, :])
                m_lo = mx[0:64, :].rearrange("p (v e) -> p v e", e=8)
                m_hi = mx_x[:, :].rearrange("p (v e) -> p v e", e=8)
                l_lo = li[0:64, :].rearrange("p (v e) -> p v e", e=8)
                l_hi = li_x[:, :].rearrange("p (v e) -> p v e", e=8)
                c2 = cm8[:, :].rearrange("p (v e) -> p v e", e=1)
                f2 = lidxf[:, :].rearrange("p (v e) -> p v e", e=1)
                nc.vector.tensor_copy(c2[:, 0:3, :], m_lo[:, 0:3, 0:1])
                nc.vector.tensor_copy(c2[:, 3:7, :], m_hi[:, 0:4, 0:1])
                nc.vector.tensor_copy(f2[:, 0:3, :], l_lo[:, 0:3, 0:1])
                nc.vector.tensor_copy(f2[:, 3:7, :], l_hi[:, 0:4, 0:1])
                g8 = work.tile([R, 8], f32, tag="g8")
                nc.vector.max(g8, cm8)
                mask = work.tile([R, 8], f32, tag="mask")
                nc.vector.tensor_scalar(mask, cm8, g8[:, 0:1], None, ALU.is_ge)
                pen = work.tile([R, 8], f32, tag="pen")
                nc.vector.tensor_scalar(pen, mask, -1.0e9, 1.0e9,
                                        ALU.mult, ALU.add)
                cand = work.tile([R, 8], f32, tag="cand")
                nc.vector.tensor_add(cand, lidxf, off_t)
                nc.vector.tensor_add(cand, cand, pen)
                nc.vector.tensor_scalar_mul(cand, cand, -1.0)
                g8b = work.tile([R, 8], f32, tag="g8b")
                nc.vector.max(g8b, cand)
                idxf = work.tile([R, 8], f32, tag="idxf")
                nc.vector.tensor_scalar_mul(idxf[:, 0:1], g8b[:, 0:1], -1.0)
                idx = work.tile([R, 8], u32, tag="idx")
                nc.vector.tensor_copy(idx[:, 0:1], idxf[:, 0:1])

                gi_cur = work.tile([R, G3], f32, tag="gi")
                nc.gpsimd.indirect_dma_start(
                    out=gi_cur, out_offset=None, in_=gi_table[:, :],
                    in_offset=bass.IndirectOffsetOnAxis(ap=idx[:, :1], axis=0))

    nc.compile()
    return nc


def _prep_inputs(inputs):
    """Host-side layout prep: transposes, padding, packing."""
    word_embs = np.asarray(inputs["word_embs"], dtype=np.float32)
    obj_feats = np.asarray(inputs["obj_feats"], dtype=np.float32)
    W_map = np.asarray(inputs["W_map"], dtype=np.float32)
    W_ih = np.asarray(inputs["W_ih"], dtype=np.float32)
    W_hh = np.asarray(inputs["W_hh"], dtype=np.float32)
    W_cls = np.asarray(inputs["W_cls"], dtype=np.float32)
    emb_table = np.asarray(inputs["emb_table"], dtype=np.float32)
    b_ih = np.asarray(inputs["b_ih"], dtype=np.float32)
    b_hh = np.asarray(inputs["b_hh"], dtype=np.float32)
    b_cls = np.asarray(inputs["b_cls"], dtype=np.float32)
    b_map = np.asarray(inputs["b_map"], dtype=np.float32)

    shared = {
        "whh": np.ascontiguousarray(W_hh.T),         # [H, 3H]
        "wcl": np.pad(W_cls, ((0, 0), (0, 7 * 512 - V))),
        "wmp": W_map,
    }

    # gi lookup table: exact x-side gate pre-activations per vocab token.
    gi_table = (emb_table.astype(np.float64) @ W_ih.T.astype(np.float64)
                + b_ih.astype(np.float64)).astype(np.float32)
    shared["gi_table"] = np.ascontiguousarray(gi_table)
    gi0_row = (word_embs[:, 0, :].astype(np.float64) @ W_ih.T.astype(np.float64)
               + b_ih.astype(np.float64)).astype(np.float32)   # [B, 3H]
    shared["gi0"] = np.ascontiguousarray(np.tile(gi0_row, (PL, 1)))
    # combine order: low chunks 0,2,4 then high 1,3,5,6, then poison.
    shared["amax_off"] = np.array(
        [[0.0, 1024.0, 2048.0, 512.0, 1536.0, 2560.0, 3072.0, 1.0e9]],
        dtype=np.float32)

    nonzero_bias = {}
    for bname, val in (("b_rz", b_hh[:2 * H]), ("b_hn", b_hh[2 * H:]),
                       ("b_cls", b_cls), ("b_map", b_map)):
        if np.any(val):
            nonzero_bias[bname] = True
            shared[bname] = np.ascontiguousarray(val[None, :], dtype=np.float32)

    in_maps = []
    for c in range(NCORES):
        m = dict(shared)
        sl = obj_feats[:, c * PL:(c + 1) * PL]           # [B, PL, F]
        objT = np.ascontiguousarray(
            np.transpose(sl, (2, 1, 0)).reshape(F, R))   # col r = pl*B + b
        m["obj"] = _pack_chunks(objT, KF)
        in_maps.append(m)
    return in_maps, nonzero_bias


TRACE = False          # test-harness hook: set True to capture an NTFF trace
LAST_RESULTS = None


def kernel(**inputs):
    global LAST_RESULTS
    in_maps, nonzero_bias = _prep_inputs(inputs)
    key = tuple(sorted(nonzero_bias))
    if key not in _CACHE:
        _CACHE[key] = _build_program(nonzero_bias)
    nc = _CACHE[key]
    res = run_bass_kernel_spmd(nc, in_maps, core_ids=list(range(NCORES)),
                               trace=TRACE)
    LAST_RESULTS = res
    full = np.empty((B, P, NSTEP, V), np.float32)
    for c in range(NCORES):
        o = res.results[c]["out"].reshape(NSTEP, PL, B, V)
        full[:, c * PL:(c + 1) * PL] = np.transpose(o, (2, 1, 0, 3))
    return full
